# revision 1
# baseline (speedup 1.0000x reference)
"""Bilinear kernel for Trainium2 (8 NeuronCores, Bass/Tile).

out[i, j] = sum_{k,l} a[i,k] * w[j,k,l] * b[i,l] + bias[j]
with B=2048, K=L=512, H=512.

Strategy: shard H (the j dim) across 8 cores (64 j's each).
Per core, for each j:
  t_j[i, k] = sum_l b[i, l] * w[j, k, l]          (tensor engine, bf16,
       4 accumulating matmuls over l-chunks of 128; stationary = b^T tile,
       moving = w_j[l, k] tile, N=512 -> one PSUM bank)
  out[i, j] = bias[j] + sum_k a[i, k] * t_j[i, k]  (DVE tensor_mul into a
       PSUM prod tile, then ScalarE activation(Copy) with accum_out for the
       free-dim sum; bias added at the end with one small DVE add per i-tile;
       `a` stays fp32)

Weights/b are rounded to bf16 (matmul runs at 1 cycle/row vs fp32's 4);
all accumulation is fp32 (PSUM + DVE reduce).
"""

import numpy as np
import ml_dtypes

N_CORES = 8
B, K, L, H = 2048, 512, 512, 512
HJ = H // N_CORES      # j's per core
P = 128                # partitions
IT = B // P            # i-tiles
LC = L // P            # l-chunks

_BF16 = ml_dtypes.bfloat16

_prog_cache = {}


def build_nc(hj=HJ, it_count=IT, reps=1, t_bufs=3, prod_bufs=3, w_bufs=3,
             split_bt=False, dve_reduce_every=0, scr_bufs=2):
    """Build the per-core Bass/Tile program (SPMD: same program, per-core data).

    reps > 1 repeats the whole compute (same inputs/outputs) for HW-timing
    differencing; only the last rep's output is DMA'd out.
    """
    import concourse.bass as bass
    import concourse.tile as tile
    from concourse import bacc, mybir

    f32 = mybir.dt.float32
    bf16 = mybir.dt.bfloat16

    nc = bacc.Bacc(trn_type="TRN2")

    # Host-prearranged layouts (see kernel() below):
    #  wt[j, p, c, k]  = w[j_global, k, c*128 + p]   (bf16)
    #  bt[p, c, i]     = b[i, c*128 + p]             (bf16)
    #  ap[p, t, k]     = a[t*128 + p, k]             (fp32)
    #  biasr[p, j]     = bias[j_global]              (fp32, replicated over p)
    wt = nc.declare_dram_parameter("wt", [hj, P, LC, K], bf16, isOutput=False)
    bt = nc.declare_dram_parameter("bt", [P, LC, B], bf16, isOutput=False)
    ap = nc.declare_dram_parameter("ap", [P, it_count, K], f32, isOutput=False)
    biasr = nc.declare_dram_parameter("biasr", [P, hj], f32, isOutput=False)
    out = nc.declare_dram_parameter("out", [it_count, P, hj], f32, isOutput=True)

    with tile.TileContext(nc) as tc:
        with (
            tc.tile_pool(name="resident", bufs=1) as res_pool,
            tc.tile_pool(name="wpool", bufs=w_bufs) as wpool,
            tc.tile_pool(name="scratch", bufs=scr_bufs) as scratch_pool,
            tc.tile_pool(name="psum", bufs=t_bufs, space=bass.MemorySpace.PSUM)
            as psum_pool,
            tc.tile_pool(name="psum_prod", bufs=prod_bufs,
                         space=bass.MemorySpace.PSUM) as prod_pool,
        ):
            # bt + the w stream go on the sync HWDGE ring; a/bias go on the
            # scalar ring so the 4MB a load doesn't delay the first matmuls.
            if split_bt:
                bt_chunks = []
                for c in range(LC):
                    t = res_pool.tile([P, B], bf16, tag=f"btc{c}", name=f"btc{c}")
                    nc.sync.dma_start(out=t[:], in_=bt[:, c, :])
                    bt_chunks.append(t)

                def bt_slice(c, it):
                    return bt_chunks[c][:, it * P:(it + 1) * P]
            else:
                bt_sb = res_pool.tile([P, LC, B], bf16, tag="bt")
                nc.sync.dma_start(out=bt_sb[:], in_=bt[:])

                def bt_slice(c, it):
                    return bt_sb[:, c, it * P:(it + 1) * P]
            a_sb = res_pool.tile([P, it_count, K], f32, tag="a")
            nc.scalar.dma_start(out=a_sb[:], in_=ap[:])
            bias_sb = res_pool.tile([P, hj], f32, tag="bias")
            nc.scalar.dma_start(out=bias_sb[:], in_=biasr[:])

            out_sb = []
            for it in range(it_count):
                out_sb.append(
                    res_pool.tile([P, hj], f32, tag=f"out{it}", name=f"out{it}")
                )

            for rep in range(reps):

                for j in range(hj):
                    w_sb = wpool.tile([P, LC, K], bf16, tag="w", name=f"w{rep}_{j}")
                    nc.sync.dma_start(out=w_sb[:], in_=wt[j])
                    for it in range(it_count):
                        t_ps = psum_pool.tile([P, K], f32, tag="t", name=f"t{rep}_{j}_{it}")
                        for c in range(LC):
                            nc.tensor.matmul(
                                t_ps[:],
                                bt_slice(c, it),
                                w_sb[:, c, :],
                                start=(c == 0),
                                stop=(c == LC - 1),
                            )
                        prod = prod_pool.tile([P, K], f32, tag="prod", name=f"p{rep}_{j}_{it}")
                        nc.vector.tensor_mul(prod[:], t_ps[:], a_sb[:, it, :])
                        # Reduce over k: mostly on ScalarE (activation accum),
                        # every Nth tile on the Vector engine to balance load.
                        seq = j * it_count + it
                        if dve_reduce_every and seq % dve_reduce_every == 0:
                            nc.vector.tensor_reduce(
                                out=out_sb[it][:, j:j + 1],
                                in_=prod[:],
                                axis=mybir.AxisListType.X,
                                op=mybir.AluOpType.add,
                            )
                        else:
                            scr = scratch_pool.tile(
                                [P, K], f32, tag="scr", name=f"s{rep}_{j}_{it}"
                            )
                            nc.scalar.activation(
                                out=scr[:],
                                in_=prod[:],
                                func=mybir.ActivationFunctionType.Copy,
                                accum_out=out_sb[it][:, j:j + 1],
                            )

                for it in range(it_count):
                    nc.vector.tensor_add(out_sb[it][:], out_sb[it][:], bias_sb[:])
                    if rep == reps - 1:
                        nc.sync.dma_start(out=out[it], in_=out_sb[it][:])

    nc.compile()
    return nc


def prep_inputs(a, b, weight, bias):
    """Host-side sharding + layout. Returns in_maps (one dict per core)."""
    a = np.asarray(a, dtype=np.float32)
    b = np.asarray(b, dtype=np.float32)
    weight = np.asarray(weight, dtype=np.float32)
    bias = np.asarray(bias, dtype=np.float32)

    # wt[j, p, c, k] = w[j, k, c*128+p]  (cast to bf16 first: halves copy volume)
    wt = weight.astype(_BF16).transpose(0, 2, 1)    # [H, L, K]
    wt = wt.reshape(H, LC, P, K)                    # [H, c, p, K]
    wt = np.ascontiguousarray(wt.transpose(0, 2, 1, 3))  # [H, p, c, K]

    # bt[p, c, i] = b[i, c*128+p]
    bt = b.T.reshape(LC, P, B).transpose(1, 0, 2)   # [p, c, i]
    bt = np.ascontiguousarray(bt).astype(_BF16)

    # ap[p, t, k] = a[t*128+p, k]
    apm = np.ascontiguousarray(a.reshape(IT, P, K).transpose(1, 0, 2))

    in_maps = []
    for c in range(N_CORES):
        jlo, jhi = c * HJ, (c + 1) * HJ
        in_maps.append({
            "wt": np.ascontiguousarray(wt[jlo:jhi]),
            "bt": bt,
            "ap": apm,
            "biasr": np.ascontiguousarray(
                np.broadcast_to(bias[jlo:jhi][None, :], (P, HJ))
            ),
        })
    return in_maps


def gather_output(results):
    """results: list (per core) of {"out": [IT, P, HJ] f32} -> [B, H] f32."""
    cols = []
    for c in range(N_CORES):
        o = np.asarray(results[c]["out"])         # [IT, P, HJ]
        cols.append(o.reshape(B, HJ))
    return np.concatenate(cols, axis=1)


def kernel(a, b, weight, bias):
    import time
    from concourse.bass_utils import run_bass_kernel_spmd

    if "nc" not in _prog_cache:
        _prog_cache["nc"] = build_nc()
    nc = _prog_cache["nc"]

    in_maps = prep_inputs(a, b, weight, bias)
    last_err = None
    for attempt in range(3):
        try:
            results = run_bass_kernel_spmd(
                nc, in_maps, core_ids=list(range(N_CORES))
            ).results
            return gather_output(results)
        except Exception as e:  # transient device/relay failures
            last_err = e
            time.sleep(10 * (attempt + 1))
    raise last_err



# revision 2
# speedup vs baseline: 1.3157x; 1.3157x over previous
"""Bilinear kernel for Trainium2 (8 NeuronCores, Bass/Tile) — mixed fp16/fp8.

out[i, j] = sum_{k,l} a[i,k] * w[j,k,l] * b[i,l] + bias[j]
with B=2048, K=L=H=512.

Sharding: H across 8 cores (HJ=64 j's each). Per core, per j, per i-tile:
  t_j[i, k] = sum_l b[i, l] * w[j, k, l]   (tensor engine)
  out[i, j] = bias[j] + sum_k a[i, k] * t_j[i, k]
     (DVE tensor_mul into a PSUM prod tile, then ScalarE
      activation(Copy, scale) with accum_out reducing over k into the
      out column — scale undoes the fp8 pre-scaling; bias added per
      i-tile at the end. NOTE: tensor_tensor_reduce with a PSUM input
      crashes the device (NRT_EXEC_UNIT_UNRECOVERABLE, isolated via
      dr_bench.py), so stage 2 stays on the two-instruction pipeline.)

Precision: NJ8 of the HJ j's per core run stage 1 in fp8 e4m3 with
perf_mode=DoubleRow — 2 matmuls of K=256 instead of 4 fp16 matmuls of
K=128 (~1.8x PE throughput on those tiles; rel-err contribution
sqrt(NJ8/HJ)*0.032 stays under the 2e-2 budget). The rest use fp16
(same PE speed as bf16, 4x less rounding noise). fp8 operands are
pre-scaled on the host (w by 2^10, b by 2^4) to dodge subnormals; the
2^-14 is undone by the ScalarE activation scale. All accumulation fp32.

Scheduling: j's are processed in groups of (1 fp8 + ~2 fp16) with the
i-tile loop INSIDE the group, so fp8 tiles (~490ns of PE work) interleave
with fp16 tiles (~860ns) and the DVE (~700ns/tile) never becomes the
local bottleneck, which it would be over a run of consecutive fp8 tiles.
(An i-tile-pairing variant that fuses two tiles per DVE op measured
~80us SLOWER in an interleaved A/B — kept behind pair_it=False.)
"""

import numpy as np
import ml_dtypes

N_CORES = 8
B, K, L, H = 2048, 512, 512, 512
HJ = H // N_CORES      # j's per core
P = 128                # partitions
IT = B // P            # i-tiles
LC = L // P            # l-chunks (fp16 path)
NJ8 = 24               # j's per core on the fp8 DoubleRow path

SW = 1024.0            # fp8 w scale (2^10)
SB = 16.0              # fp8 b scale (2^4)
INV_S = 1.0 / (SW * SB)

_F16 = np.float16
_E4 = ml_dtypes.float8_e4m3    # TRN fp8e4: max normal 240

_prog_cache = {}


def fp8_j_set(nj8=NJ8):
    """Spread nj8 fp8-j's evenly across the HJ j-indices."""
    if nj8 <= 0:
        return set()
    s = {round(i * HJ / nj8) for i in range(nj8)}
    assert len(s) == nj8
    return s


def j_groups(nj8=NJ8, hj=HJ):
    """Groups of j's processed concurrently (i-tile loop inside the group):
    one fp8 j per group plus the fp16 j's dealt round-robin."""
    js8 = sorted(fp8_j_set(nj8))
    js16 = [j for j in range(hj) if j not in set(js8)]
    if not js8:
        return [[j] for j in js16]
    groups = [[j] for j in js8]
    for i, j in enumerate(js16):
        groups[i % len(groups)].append(j)
    # put the fp8 j in the middle of its group
    out = []
    for g in groups:
        mid = len(g) // 2
        out.append(g[1:mid + 1] + [g[0]] + g[mid + 1:])
    return out


def build_nc(hj=HJ, it_count=IT, reps=1, nj8=NJ8, t_bufs=None, prod_bufs=None,
             w_bufs=5, w8_bufs=3, scr_bufs=2, hw_loop=False, pair_it=False):
    """Per-core Bass/Tile program (SPMD: same program, per-core data).

    reps>1 repeats the whole compute for HW-timing differencing. With
    hw_loop=True the repetition is a hardware For_i loop (no instruction
    bloat — body emitted once), so reps can be large (e.g. 129) to
    swamp the ~7ms axon-relay call noise."""
    import concourse.bass as bass
    import concourse.tile as tile
    from concourse import bacc, mybir

    f32 = mybir.dt.float32
    f16 = mybir.dt.float16
    f8 = mybir.dt.float8e4

    if t_bufs is None:
        t_bufs = 2 if pair_it else 4
    if prod_bufs is None:
        prod_bufs = 2 if pair_it else 3

    js8 = fp8_j_set(nj8)
    n8 = len(js8)
    n16 = hj - n8
    groups = j_groups(nj8, hj)

    nc = bacc.Bacc(trn_type="TRN2")

    # Host-prearranged layouts (see prep_inputs):
    #  wt16[jj, p, c, k]    = w[j, k, 128c + p]              (fp16, fp16-j's)
    #  wt8[jj, p, c2, h, k] = w[j, k, 256c2 + 128h + p]*SW   (fp8, fp8-j's)
    #  bt16[p, c, i]        = b[i, 128c + p]                 (fp16)
    #  bt8[p, c2, h, i]     = b[i, 256c2 + 128h + p]*SB      (fp8)
    #  ap[p, t, k]          = a[128t + p, k]                 (fp32)
    #  biasr[p, jj]         = bias[j_global]                 (fp32, replicated)
    # jj indexes within the dtype-class in GROUP EMISSION ORDER.
    wt16 = nc.declare_dram_parameter("wt16", [max(n16, 1), P, LC, K], f16,
                                     isOutput=False)
    wt8 = nc.declare_dram_parameter("wt8", [max(n8, 1), P, 2, 2, K], f8,
                                    isOutput=False)
    bt16 = nc.declare_dram_parameter("bt16", [P, LC, B], f16, isOutput=False)
    bt8 = nc.declare_dram_parameter("bt8", [P, 2, 2, B], f8, isOutput=False)
    ap = nc.declare_dram_parameter("ap", [P, it_count, K], f32, isOutput=False)
    biasr = nc.declare_dram_parameter("biasr", [P, hj], f32, isOutput=False)
    out = nc.declare_dram_parameter("out", [it_count, P, hj], f32, isOutput=True)

    with tile.TileContext(nc) as tc:
        with (
            tc.tile_pool(name="resident", bufs=1) as res_pool,
            tc.tile_pool(name="wpool", bufs=w_bufs) as wpool,
            tc.tile_pool(name="w8pool", bufs=w8_bufs) as w8pool,
            tc.tile_pool(name="scratch", bufs=scr_bufs) as scratch_pool,
            tc.tile_pool(name="psum", bufs=t_bufs, space=bass.MemorySpace.PSUM)
            as psum_pool,
            tc.tile_pool(name="psum_prod", bufs=prod_bufs,
                         space=bass.MemorySpace.PSUM) as prod_pool,
        ):
            # Spread resident loads across DMA rings (only SP/ACT/gpsimd can
            # initiate DMAs) so the w stream on the sync ring starts
            # immediately and the first matmul only waits on bt16.
            bt16_sb = res_pool.tile([P, LC, B], f16, tag="bt16")
            nc.gpsimd.dma_start(out=bt16_sb[:], in_=bt16[:])
            bt8_sb = res_pool.tile([P, 2, 2, B], f8, tag="bt8")
            nc.scalar.dma_start(out=bt8_sb[:], in_=bt8[:])
            a_sb = res_pool.tile([P, it_count, K], f32, tag="a")
            nc.scalar.dma_start(out=a_sb[:], in_=ap[:])
            bias_sb = res_pool.tile([P, hj], f32, tag="bias")
            nc.scalar.dma_start(out=bias_sb[:], in_=biasr[:])

            out_sb = []
            for it in range(it_count):
                out_sb.append(
                    res_pool.tile([P, hj], f32, tag=f"out{it}", name=f"out{it}")
                )

            def emit_body(rep, dma_out):
                i16 = i8 = 0
                for g, group in enumerate(groups):
                    w_tiles = {}
                    for j in group:
                        if j in js8:
                            w_sb = w8pool.tile([P, 2, 2, K], f8, tag="w8",
                                               name=f"w8_{rep}_{j}")
                            nc.sync.dma_start(out=w_sb[:], in_=wt8[i8])
                            i8 += 1
                        else:
                            w_sb = wpool.tile([P, LC, K], f16, tag="w",
                                              name=f"w{rep}_{j}")
                            nc.sync.dma_start(out=w_sb[:], in_=wt16[i16])
                            i16 += 1
                        w_tiles[j] = w_sb
                    def emit_matmuls(t_dst, j, it, is8, w_sb):
                        if is8:
                            for c2 in range(2):
                                nc.tensor.matmul(
                                    t_dst,
                                    bt8_sb[:, c2, :, it * P:(it + 1) * P],
                                    w_sb[:, c2, :, :],
                                    start=(c2 == 0),
                                    stop=(c2 == 1),
                                    perf_mode=mybir.MatmulPerfMode.DoubleRow,
                                )
                        else:
                            for c in range(LC):
                                nc.tensor.matmul(
                                    t_dst,
                                    bt16_sb[:, c, it * P:(it + 1) * P],
                                    w_sb[:, c, :],
                                    start=(c == 0),
                                    stop=(c == LC - 1),
                                )

                    if pair_it:
                        # i-tiles in PAIRS: both halves of a [P, 2, K] PSUM
                        # tile (2 banks) filled by separate accumulation
                        # groups, then ONE DVE tensor_mul covers both
                        # (amortizes PSUM access + seq overhead).
                        for itp in range(0, it_count, 2):
                            for j in group:
                                is8 = j in js8
                                w_sb = w_tiles[j]
                                t_ps = psum_pool.tile(
                                    [P, 2, K], f32, tag="t",
                                    name=f"t{rep}_{j}_{itp}")
                                for h in range(2):
                                    emit_matmuls(t_ps[:, h, :], j, itp + h,
                                                 is8, w_sb)
                                prod = prod_pool.tile(
                                    [P, 2, K], f32, tag="prod",
                                    name=f"p{rep}_{j}_{itp}")
                                nc.vector.tensor_mul(
                                    prod[:], t_ps[:], a_sb[:, itp:itp + 2, :])
                                for h in range(2):
                                    scr = scratch_pool.tile(
                                        [P, K], f32, tag="scr",
                                        name=f"s{rep}_{j}_{itp}_{h}")
                                    nc.scalar.activation(
                                        out=scr[:],
                                        in_=prod[:, h, :],
                                        func=mybir.ActivationFunctionType.Copy,
                                        scale=(INV_S if is8 else 1.0),
                                        accum_out=out_sb[itp + h][:, j:j + 1],
                                    )
                    else:
                        for it in range(it_count):
                            for j in group:
                                is8 = j in js8
                                w_sb = w_tiles[j]
                                t_ps = psum_pool.tile(
                                    [P, K], f32, tag="t",
                                    name=f"t{rep}_{j}_{it}")
                                emit_matmuls(t_ps[:], j, it, is8, w_sb)
                                prod = prod_pool.tile(
                                    [P, K], f32, tag="prod",
                                    name=f"p{rep}_{j}_{it}")
                                nc.vector.tensor_mul(prod[:], t_ps[:],
                                                     a_sb[:, it, :])
                                scr = scratch_pool.tile(
                                    [P, K], f32, tag="scr",
                                    name=f"s{rep}_{j}_{it}")
                                nc.scalar.activation(
                                    out=scr[:],
                                    in_=prod[:],
                                    func=mybir.ActivationFunctionType.Copy,
                                    scale=(INV_S if is8 else 1.0),
                                    accum_out=out_sb[it][:, j:j + 1],
                                )

                for it in range(it_count):
                    nc.vector.tensor_add(out_sb[it][:], out_sb[it][:],
                                         bias_sb[:])
                    if dma_out:
                        nc.sync.dma_start(out=out[it], in_=out_sb[it][:])

            if hw_loop and reps > 1:
                with tc.For_i(0, reps, 1):
                    emit_body(0, dma_out=True)
            else:
                for rep in range(reps):
                    emit_body(rep, dma_out=(rep == reps - 1))

    nc.compile()
    return nc


def prep_inputs(a, b, weight, bias, nj8=NJ8):
    """Host-side sharding + layout. Returns in_maps (one dict per core)."""
    a = np.asarray(a, dtype=np.float32)
    b = np.asarray(b, dtype=np.float32)
    weight = np.asarray(weight, dtype=np.float32)
    bias = np.asarray(bias, dtype=np.float32)

    groups = j_groups(nj8)
    js8 = fp8_j_set(nj8)
    # emission order within each dtype class
    order16 = [j for g in groups for j in g if j not in js8]
    order8 = [j for g in groups for j in g if j in js8]

    # fp16 weights: wt16[j, p, c, k] = w[j, k, 128c + p]
    w16 = weight.astype(_F16).transpose(0, 2, 1)         # [H, L, K]
    w16 = w16.reshape(H, LC, P, K).transpose(0, 2, 1, 3)  # [H, p, c, K]

    # fp8 weights: wt8[j, p, c2, h, k] = w[j, k, 256c2 + 128h + p] * SW
    w8 = np.clip(weight * SW, -240, 240).astype(_E4).transpose(0, 2, 1)
    w8 = w8.reshape(H, 2, 2, P, K).transpose(0, 3, 1, 2, 4)  # [H, p, c2, h, K]

    # bt16[p, c, i] = b[i, 128c + p]
    bt16 = b.T.reshape(LC, P, B).transpose(1, 0, 2)
    bt16 = np.ascontiguousarray(bt16).astype(_F16)

    # bt8[p, c2, h, i] = b[i, 256c2 + 128h + p] * SB
    bt8 = np.clip(b * SB, -240, 240).astype(_E4).T.reshape(2, 2, P, B)
    bt8 = np.ascontiguousarray(bt8.transpose(2, 0, 1, 3))

    # ap[p, t, k] = a[128t + p, k]
    apm = np.ascontiguousarray(a.reshape(IT, P, K).transpose(1, 0, 2))

    in_maps = []
    for c in range(N_CORES):
        jlo = c * HJ
        j16 = [jlo + j for j in order16]
        j8 = [jlo + j for j in order8]
        in_maps.append({
            "wt16": np.ascontiguousarray(w16[j16]) if j16 else
            np.zeros((1, P, LC, K), _F16),
            "wt8": np.ascontiguousarray(w8[j8]) if j8 else
            np.zeros((1, P, 2, 2, K), _E4),
            "bt16": bt16,
            "bt8": bt8,
            "ap": apm,
            "biasr": np.ascontiguousarray(
                np.broadcast_to(bias[jlo:jlo + HJ][None, :], (P, HJ))
            ),
        })
    return in_maps


def gather_output(results, nj8=NJ8):
    """results: per-core {"out": [IT, P, HJ] f32} -> [B, H] f32.

    out columns are indexed by local j directly (accum_out targets column
    j), so no reordering is needed."""
    cols = []
    for c in range(N_CORES):
        o = np.asarray(results[c]["out"])         # [IT, P, HJ]
        cols.append(o.reshape(B, HJ))
    return np.concatenate(cols, axis=1)


def kernel(a, b, weight, bias):
    import time
    from concourse.bass_utils import run_bass_kernel_spmd

    if "nc" not in _prog_cache:
        _prog_cache["nc"] = build_nc()
    nc = _prog_cache["nc"]

    in_maps = prep_inputs(a, b, weight, bias)
    last_err = None
    for attempt in range(3):
        try:
            results = run_bass_kernel_spmd(
                nc, in_maps, core_ids=list(range(N_CORES))
            ).results
            return gather_output(results)
        except Exception as e:  # transient device/relay failures
            last_err = e
            time.sleep(10 * (attempt + 1))
    raise last_err


# revision 3
# speedup vs baseline: 1.3666x; 1.0388x over previous
"""Bilinear kernel for Trainium2 (8 NeuronCores, Bass/Tile) — mixed fp16/fp8.

out[i, j] = sum_{k,l} a[i,k] * w[j,k,l] * b[i,l] + bias[j]
with B=2048, K=L=H=512.

Sharding: H across 8 cores (HJ=64 j's each). Per core, per j, per i-tile:
  t_j[i, k] = sum_l b[i, l] * w[j, k, l]   (tensor engine)
  out[i, j] = bias[j] + sum_k a[i, k] * t_j[i, k]
     (DVE tensor_mul into a PSUM prod tile, then ScalarE
      activation(Copy, scale) with accum_out reducing over k into the
      out column — scale undoes the fp8 pre-scaling; bias added per
      i-tile at the end. NOTE: tensor_tensor_reduce with a PSUM input
      crashes the device (NRT_EXEC_UNIT_UNRECOVERABLE, isolated via
      dr_bench.py), so stage 2 stays on the two-instruction pipeline.)

Precision: NJ8 of the HJ j's per core run stage 1 in fp8 e4m3 with
perf_mode=DoubleRow — 2 matmuls of K=256 instead of 4 fp16 matmuls of
K=128 (~1.8x PE throughput on those tiles; rel-err contribution
sqrt(NJ8/HJ)*0.032 stays under the 2e-2 budget). The rest use fp16
(same PE speed as bf16, 4x less rounding noise). fp8 operands are
pre-scaled on the host (w by 2^10, b by 2^4) to dodge subnormals; the
2^-14 is undone by the ScalarE activation scale. All accumulation fp32.

Scheduling: j's are processed in groups of (1 fp8 + ~2 fp16) with the
i-tile loop INSIDE the group, so fp8 tiles (~490ns of PE work) interleave
with fp16 tiles (~860ns) and the DVE (~700ns/tile) never becomes the
local bottleneck, which it would be over a run of consecutive fp8 tiles.
(An i-tile-pairing variant that fuses two tiles per DVE op measured
~80us SLOWER in an interleaved A/B — kept behind pair_it=False.)
"""

import numpy as np
import ml_dtypes

N_CORES = 8
B, K, L, H = 2048, 512, 512, 512
HJ = H // N_CORES      # j's per core
P = 128                # partitions
IT = B // P            # i-tiles
LC = L // P            # l-chunks (fp16 path)
NJ8 = 24               # j's per core on the fp8 DoubleRow path

SW = 1024.0            # fp8 w scale (2^10)
SB = 16.0              # fp8 b scale (2^4)
INV_S = 1.0 / (SW * SB)

_F16 = np.float16
_E4 = ml_dtypes.float8_e4m3    # TRN fp8e4: max normal 240

_prog_cache = {}


def fp8_j_set(nj8=NJ8):
    """Spread nj8 fp8-j's evenly across the HJ j-indices."""
    if nj8 <= 0:
        return set()
    s = {round(i * HJ / nj8) for i in range(nj8)}
    assert len(s) == nj8
    return s


def j_groups(nj8=NJ8, hj=HJ):
    """Groups of j's processed concurrently (i-tile loop inside the group):
    one fp8 j per group plus the fp16 j's dealt round-robin."""
    js8 = sorted(fp8_j_set(nj8))
    js16 = [j for j in range(hj) if j not in set(js8)]
    if not js8:
        return [[j] for j in js16]
    groups = [[j] for j in js8]
    for i, j in enumerate(js16):
        groups[i % len(groups)].append(j)
    # put the fp8 j in the middle of its group
    out = []
    for g in groups:
        mid = len(g) // 2
        out.append(g[1:mid + 1] + [g[0]] + g[mid + 1:])
    return out


def build_nc(hj=HJ, it_count=IT, reps=1, nj8=NJ8, t_bufs=None, prod_bufs=None,
             w_bufs=5, w8_bufs=3, scr_bufs=2, hw_loop=False, pair_it=False):
    """Per-core Bass/Tile program (SPMD: same program, per-core data).

    reps>1 repeats the whole compute for HW-timing differencing. With
    hw_loop=True the repetition is a hardware For_i loop (no instruction
    bloat — body emitted once), so reps can be large (e.g. 129) to
    swamp the ~7ms axon-relay call noise."""
    import concourse.bass as bass
    import concourse.tile as tile
    from concourse import bacc, mybir

    f32 = mybir.dt.float32
    f16 = mybir.dt.float16
    f8 = mybir.dt.float8e4

    if t_bufs is None:
        t_bufs = 2 if pair_it else 3
    if prod_bufs is None:
        prod_bufs = 2 if pair_it else 3

    js8 = fp8_j_set(nj8)
    n8 = len(js8)
    n16 = hj - n8
    groups = j_groups(nj8, hj)

    nc = bacc.Bacc(trn_type="TRN2")

    # Host-prearranged layouts (see prep_inputs):
    #  wt16[jj, p, c, k]    = w[j, k, 128c + p]              (fp16, fp16-j's)
    #  wt8[jj, p, c2, h, k] = w[j, k, 256c2 + 128h + p]*SW   (fp8, fp8-j's)
    #  bt16[p, c, i]        = b[i, 128c + p]                 (fp16)
    #  bt8[p, c2, h, i]     = b[i, 256c2 + 128h + p]*SB      (fp8)
    #  ap[p, t, k]          = a[128t + p, k]                 (fp32)
    #  biasr[p, jj]         = bias[j_global]                 (fp32, replicated)
    # jj indexes within the dtype-class in GROUP EMISSION ORDER.
    wt16 = nc.declare_dram_parameter("wt16", [max(n16, 1), P, LC, K], f16,
                                     isOutput=False)
    wt8 = nc.declare_dram_parameter("wt8", [max(n8, 1), P, 2, 2, K], f8,
                                    isOutput=False)
    bt16 = nc.declare_dram_parameter("bt16", [P, LC, B], f16, isOutput=False)
    bt8 = nc.declare_dram_parameter("bt8", [P, 2, 2, B], f8, isOutput=False)
    ap = nc.declare_dram_parameter("ap", [P, it_count, K], f32, isOutput=False)
    biasr = nc.declare_dram_parameter("biasr", [P, hj], f32, isOutput=False)
    out = nc.declare_dram_parameter("out", [it_count, P, hj], f32, isOutput=True)

    with tile.TileContext(nc) as tc:
        with (
            tc.tile_pool(name="resident", bufs=1) as res_pool,
            tc.tile_pool(name="wpool", bufs=w_bufs) as wpool,
            tc.tile_pool(name="w8pool", bufs=w8_bufs) as w8pool,
            tc.tile_pool(name="scratch", bufs=scr_bufs) as scratch_pool,
            tc.tile_pool(name="psum", bufs=t_bufs, space=bass.MemorySpace.PSUM)
            as psum_pool,
            tc.tile_pool(name="psum_prod", bufs=prod_bufs,
                         space=bass.MemorySpace.PSUM) as prod_pool,
        ):
            # Spread resident loads across DMA rings (only SP/ACT/gpsimd can
            # initiate DMAs) so the w stream on the sync ring starts
            # immediately and the first matmul only waits on bt16.
            bt16_sb = res_pool.tile([P, LC, B], f16, tag="bt16")
            nc.gpsimd.dma_start(out=bt16_sb[:], in_=bt16[:])
            bt8_sb = res_pool.tile([P, 2, 2, B], f8, tag="bt8")
            nc.scalar.dma_start(out=bt8_sb[:], in_=bt8[:])
            a_sb = res_pool.tile([P, it_count, K], f32, tag="a")
            nc.scalar.dma_start(out=a_sb[:], in_=ap[:])
            bias_sb = res_pool.tile([P, hj], f32, tag="bias")
            nc.scalar.dma_start(out=bias_sb[:], in_=biasr[:])

            out_sb = []
            for it in range(it_count):
                out_sb.append(
                    res_pool.tile([P, hj], f32, tag=f"out{it}", name=f"out{it}")
                )

            def emit_body(rep, dma_out):
                i16 = i8 = 0
                for g, group in enumerate(groups):
                    w_tiles = {}
                    for j in group:
                        if j in js8:
                            w_sb = w8pool.tile([P, 2, 2, K], f8, tag="w8",
                                               name=f"w8_{rep}_{j}")
                            nc.sync.dma_start(out=w_sb[:], in_=wt8[i8])
                            i8 += 1
                        else:
                            w_sb = wpool.tile([P, LC, K], f16, tag="w",
                                              name=f"w{rep}_{j}")
                            nc.sync.dma_start(out=w_sb[:], in_=wt16[i16])
                            i16 += 1
                        w_tiles[j] = w_sb
                    def emit_matmuls(t_dst, j, it, is8, w_sb):
                        if is8:
                            for c2 in range(2):
                                nc.tensor.matmul(
                                    t_dst,
                                    bt8_sb[:, c2, :, it * P:(it + 1) * P],
                                    w_sb[:, c2, :, :],
                                    start=(c2 == 0),
                                    stop=(c2 == 1),
                                    perf_mode=mybir.MatmulPerfMode.DoubleRow,
                                )
                        else:
                            for c in range(LC):
                                nc.tensor.matmul(
                                    t_dst,
                                    bt16_sb[:, c, it * P:(it + 1) * P],
                                    w_sb[:, c, :],
                                    start=(c == 0),
                                    stop=(c == LC - 1),
                                )

                    if pair_it:
                        # i-tiles in PAIRS: both halves of a [P, 2, K] PSUM
                        # tile (2 banks) filled by separate accumulation
                        # groups, then ONE DVE tensor_mul covers both
                        # (amortizes PSUM access + seq overhead).
                        for itp in range(0, it_count, 2):
                            for j in group:
                                is8 = j in js8
                                w_sb = w_tiles[j]
                                t_ps = psum_pool.tile(
                                    [P, 2, K], f32, tag="t",
                                    name=f"t{rep}_{j}_{itp}")
                                for h in range(2):
                                    emit_matmuls(t_ps[:, h, :], j, itp + h,
                                                 is8, w_sb)
                                prod = prod_pool.tile(
                                    [P, 2, K], f32, tag="prod",
                                    name=f"p{rep}_{j}_{itp}")
                                nc.vector.tensor_mul(
                                    prod[:], t_ps[:], a_sb[:, itp:itp + 2, :])
                                for h in range(2):
                                    scr = scratch_pool.tile(
                                        [P, K], f32, tag="scr",
                                        name=f"s{rep}_{j}_{itp}_{h}")
                                    nc.scalar.activation(
                                        out=scr[:],
                                        in_=prod[:, h, :],
                                        func=mybir.ActivationFunctionType.Copy,
                                        scale=(INV_S if is8 else 1.0),
                                        accum_out=out_sb[itp + h][:, j:j + 1],
                                    )
                    else:
                        for it in range(it_count):
                            for j in group:
                                is8 = j in js8
                                w_sb = w_tiles[j]
                                t_ps = psum_pool.tile(
                                    [P, K], f32, tag="t",
                                    name=f"t{rep}_{j}_{it}")
                                emit_matmuls(t_ps[:], j, it, is8, w_sb)
                                prod = prod_pool.tile(
                                    [P, K], f32, tag="prod",
                                    name=f"p{rep}_{j}_{it}")
                                nc.vector.tensor_mul(prod[:], t_ps[:],
                                                     a_sb[:, it, :])
                                scr = scratch_pool.tile(
                                    [P, K], f32, tag="scr",
                                    name=f"s{rep}_{j}_{it}")
                                nc.scalar.activation(
                                    out=scr[:],
                                    in_=prod[:],
                                    func=mybir.ActivationFunctionType.Copy,
                                    scale=(INV_S if is8 else 1.0),
                                    accum_out=out_sb[it][:, j:j + 1],
                                )

                for it in range(it_count):
                    nc.vector.tensor_add(out_sb[it][:], out_sb[it][:],
                                         bias_sb[:])
                    if dma_out:
                        nc.sync.dma_start(out=out[it], in_=out_sb[it][:])

            if hw_loop and reps > 1:
                with tc.For_i(0, reps, 1):
                    emit_body(0, dma_out=True)
            else:
                for rep in range(reps):
                    emit_body(rep, dma_out=(rep == reps - 1))

    nc.compile()
    return nc


def prep_inputs(a, b, weight, bias, nj8=NJ8):
    """Host-side sharding + layout. Returns in_maps (one dict per core)."""
    a = np.asarray(a, dtype=np.float32)
    b = np.asarray(b, dtype=np.float32)
    weight = np.asarray(weight, dtype=np.float32)
    bias = np.asarray(bias, dtype=np.float32)

    groups = j_groups(nj8)
    js8 = fp8_j_set(nj8)
    # emission order within each dtype class
    order16 = [j for g in groups for j in g if j not in js8]
    order8 = [j for g in groups for j in g if j in js8]

    # fp16 weights: wt16[j, p, c, k] = w[j, k, 128c + p]
    w16 = weight.astype(_F16).transpose(0, 2, 1)         # [H, L, K]
    w16 = w16.reshape(H, LC, P, K).transpose(0, 2, 1, 3)  # [H, p, c, K]

    # fp8 weights: wt8[j, p, c2, h, k] = w[j, k, 256c2 + 128h + p] * SW
    w8 = np.clip(weight * SW, -240, 240).astype(_E4).transpose(0, 2, 1)
    w8 = w8.reshape(H, 2, 2, P, K).transpose(0, 3, 1, 2, 4)  # [H, p, c2, h, K]

    # bt16[p, c, i] = b[i, 128c + p]
    bt16 = b.T.reshape(LC, P, B).transpose(1, 0, 2)
    bt16 = np.ascontiguousarray(bt16).astype(_F16)

    # bt8[p, c2, h, i] = b[i, 256c2 + 128h + p] * SB
    bt8 = np.clip(b * SB, -240, 240).astype(_E4).T.reshape(2, 2, P, B)
    bt8 = np.ascontiguousarray(bt8.transpose(2, 0, 1, 3))

    # ap[p, t, k] = a[128t + p, k]
    apm = np.ascontiguousarray(a.reshape(IT, P, K).transpose(1, 0, 2))

    in_maps = []
    for c in range(N_CORES):
        jlo = c * HJ
        j16 = [jlo + j for j in order16]
        j8 = [jlo + j for j in order8]
        in_maps.append({
            "wt16": np.ascontiguousarray(w16[j16]) if j16 else
            np.zeros((1, P, LC, K), _F16),
            "wt8": np.ascontiguousarray(w8[j8]) if j8 else
            np.zeros((1, P, 2, 2, K), _E4),
            "bt16": bt16,
            "bt8": bt8,
            "ap": apm,
            "biasr": np.ascontiguousarray(
                np.broadcast_to(bias[jlo:jlo + HJ][None, :], (P, HJ))
            ),
        })
    return in_maps


def gather_output(results, nj8=NJ8):
    """results: per-core {"out": [IT, P, HJ] f32} -> [B, H] f32.

    out columns are indexed by local j directly (accum_out targets column
    j), so no reordering is needed."""
    cols = []
    for c in range(N_CORES):
        o = np.asarray(results[c]["out"])         # [IT, P, HJ]
        cols.append(o.reshape(B, HJ))
    return np.concatenate(cols, axis=1)


def kernel(a, b, weight, bias):
    import time
    from concourse.bass_utils import run_bass_kernel_spmd

    if "nc" not in _prog_cache:
        _prog_cache["nc"] = build_nc()
    nc = _prog_cache["nc"]

    in_maps = prep_inputs(a, b, weight, bias)
    last_err = None
    for attempt in range(3):
        try:
            results = run_bass_kernel_spmd(
                nc, in_maps, core_ids=list(range(N_CORES))
            ).results
            return gather_output(results)
        except Exception as e:  # transient device/relay failures
            last_err = e
            time.sleep(10 * (attempt + 1))
    raise last_err
